# revision 1
# baseline (speedup 1.0000x reference)
"""CDSSM kernel for Trainium2 (8 NeuronCores, data-parallel over batch).

Model (per sequence of T=2048, D=128):
  h1 = tanh(conv1d(x^T, w1, b1))        # [K=128,  T-2]  (FL=3, VALID)
  h2 = tanh(conv1d(h1, w2, b2))         # [K2=128, T-4]
  hmax = max_t(h2)                      # k-max pooling, k=1
  s = tanh(sw @ hmax + sb)              # [L=64]
Then dots q.s/d.s, scale by gamma, softmax over [pos, n0, n1].

Sharding: B=64 split 8 ways; each core encodes 32 sequences
(8 q with q-weights, 8 pos + 16 negs with d-weights).
Convs run as PE matmuls (3 taps accumulated in PSUM, fp32r operands at
N=512 so the PE streams 1 col/cycle) on a host-pre-transposed [D, T] slab.

Walrus constraint: self-loading fp32/fp32r matmuls carry at most ONE sync
wait. All PSUM results are consumed by ScalarE (tanh) so conv matmul deps
collapse onto the single ACT semaphore; two warm-up matmuls at kernel start
absorb the weight-DMA queue waits.
"""

import numpy as np

B, T, D = 64, 2048, 128
K, K2, L, FL, J = 128, 128, 64, 3, 2
NCORES = 8
BPC = B // NCORES          # sequences of each role per core
NSEQ = 4 * BPC             # 32 slabs per core: [q x8 | pos x8 | n0 x8 | n1 x8]
T1 = T - FL + 1            # 2046
T2 = T1 - FL + 1           # 2044
NTILE = 512                # PSUM free-dim tile

# module-level handle for test harness introspection (exec time / profile)
LAST_RESULTS = None


def _col_tiles(total):
    out = []
    c = 0
    while c < total:
        out.append((c, min(NTILE, total - c)))
        c += NTILE
    return out


def _build_program(gw_val: float, reps: int = 1):
    import concourse.bacc as bacc
    import concourse.bass as bass
    import concourse.tile as tile
    from concourse import mybir

    f32 = mybir.dt.float32
    f32r = mybir.dt.float32r
    bf16 = mybir.dt.bfloat16
    AFT = mybir.ActivationFunctionType
    AX = mybir.AxisListType

    nc = bacc.Bacc()

    def _observe(eng, ins=(), outs=()):
        # NoOp with APs: Tile wires the deps onto it, so the wait lands here
        # instead of on the next (wait-slot-limited) instruction. Declaring
        # an output makes downstream accessors order after it.
        inst = mybir.InstNoOp(
            name=nc.get_next_instruction_name(), text_hint="obs",
            bass_nofuse=True,
        )
        inst.ins = [eng.lower_ap(ap) for ap in ins]
        inst.outs = [eng.lower_ap(ap) for ap in outs]
        return eng.add_instruction(inst)

    x_d = nc.dram_tensor("x", [NSEQ, D, T], bf16, kind="ExternalInput")
    # packed consts (f32-typed; bf16 views bitcast out):
    # cols [0:128)=swT, [128:130)=b1, [130:132)=b2, [132:134)=sb (rows 64+
    # unused), [134:518)=wc1 bf16 pairs, [518:902)=wc2 bf16 pairs
    blob_d = nc.dram_tensor("blob", [128, 902], f32, kind="ExternalInput")
    out_d = nc.dram_tensor("out", [1, 3 * BPC], f32, kind="ExternalOutput")

    with tile.TileContext(nc) as tc:
        with (
            tc.tile_pool(name="consts", bufs=1) as consts,
            tc.tile_pool(name="xp", bufs=1) as xp,
            tc.tile_pool(name="h1p", bufs=1) as h1p,
            tc.tile_pool(name="mxp", bufs=2) as mxp,
            tc.tile_pool(name="smallp", bufs=1) as smallp,
            tc.tile_pool(name="ps1", bufs=2, space="PSUM") as ps1,
            tc.tile_pool(name="ps2", bufs=2, space="PSUM") as ps2,
            tc.tile_pool(name="psf", bufs=1, space="PSUM") as psf,
        ):
            # --- weights / constants in SBUF (1 SWDGE queue) ---
            blob = consts.tile([128, 902], f32)
            nc.gpsimd.dma_start(out=blob, in_=blob_d[:, :])
            wc1 = blob[:, 134:518].bitcast(bf16).rearrange(
                "d (i k) -> d i k", k=128)                    # conv1 lhsT bf16
            wc2 = blob[:, 518:902].bitcast(bf16).rearrange(
                "d (i k) -> d i k", k=128)                    # conv2 lhsT bf16
            swt = blob[:, 0:128]                              # cols 0:64 q, 64:128 d
            b1t = blob[:, 128:130]
            b2t = blob[:, 130:132]
            sbt = blob[0:L, 132:134]
            ones = consts.tile([L, 1], f32)
            nc.vector.memset(ones, 1.0)

            # warm-up matmuls: absorb the weight-DMA queue waits on PE so
            # every later self-loading matmul carries <=1 sync wait
            junk = psf.tile([1, 2], f32, tag="scratch")
            nc.tensor.matmul(junk, wc1[:, 0, 0:1], wc1[:, 0, 0:2],
                             start=True, stop=True)
            nc.tensor.matmul(junk, swt[:, 0:1].bitcast(bf16)[:, 0:1],
                             swt[:, 0:1].bitcast(bf16),
                             start=True, stop=True, skip_group_check=True)

            # zero-dep ACT table warm-ups: the first use of each LUT function
            # carries an implicit table-load sync, leaving no room for real
            # waits — preload the tables on scratch data instead
            trash = consts.tile([128, 8], f32)
            nc.vector.memset(trash, 0.0)
            nc.scalar.activation(trash[:, 0:1], trash[:, 1:2], AFT.Tanh)
            nc.scalar.activation(trash[:, 2:3], trash[:, 3:4], AFT.Exp)

            # ACT observes each bias-DMA queue once, on a throwaway copy, so
            # real activations never pair a DMA wait with their PE wait
            nc.scalar.copy(trash[:, 4:5], b1t[:, 0:1])
            nc.scalar.copy(trash[:, 5:6], b2t[:, 0:1])
            nc.scalar.copy(trash[0:L, 6:7], sbt[:, 0:1])

            H = smallp.tile([128, NSEQ], f32)     # pooled (pre-tanh) max per seq
            # persistent double-buffered h1 ring: same-tile writes avoid the
            # cross-generation WAW semaphore a pool reallocation would add
            h1full = h1p.tile([128, 2, T1], bf16)
            trashc = smallp.tile([1, 4 * NSEQ + 8], f32)   # carrier scratch
            # all 32 activation slabs resident (bf16): disjoint regions mean
            # every x-DMA is dependency-free (no slot-reuse WAR/WAW waits)
            xfull = xp.tile([128, NSEQ, T], bf16)

            c1_tiles = _col_tiles(T1)
            c2_tiles = _col_tiles(T2)

            # x loads in 4 chunks (4 HWDGE queues): small first chunk so the
            # PE can start almost immediately
            chunks = [0, 2, 8, NSEQ]
            for ci in range(3):
                a, b = chunks[ci], chunks[ci + 1]
                nc.sync.dma_start(
                    out=xfull[:, a:b, :],
                    in_=x_d[a:b].rearrange("s d t -> d s t"))

            import contextlib
            loop_ctx = (tc.For_i(0, reps, 1) if reps > 1
                        else contextlib.nullcontext())
            with loop_ctx:
              for s in range(NSEQ):
                  e = 0 if s < BPC else 1
                  xt = xfull[:, s, :]

                  if s in chunks:
                      # PE wait-carrier: absorbs the chunk-DMA wait so conv1
                      # matmuls only wait on their PSUM slot release (ACT)
                      nc.tensor.matmul(junk, xt[:, 0:1], xt[:, 0:2],
                                       start=True, stop=True,
                                       skip_group_check=True)

                  h1 = h1full[:, s % 2, :]
                  for jt1, (c0, w) in enumerate(c1_tiles):
                      ps = ps1.tile([128, NTILE], f32)
                      for f in range(FL):
                          nc.tensor.matmul(
                              ps[:, :w],
                              wc1[:, e * FL + f, :],
                              xt[:, c0 + f:c0 + f + w],
                              start=(f == 0),
                              stop=(f == FL - 1),
                          )
                      # ACT wait-carrier: takes the PE (psum-done) wait so the
                      # tanh below only carries its same-engine WAW wait
                      ci = s * 4 + jt1
                      nc.scalar.copy(trashc[0:1, ci:ci + 1], ps[0:1, 0:1])
                      nc.scalar.activation(
                          h1[:, c0:c0 + w], ps[:, :w], AFT.Tanh,
                          bias=b1t[:, e:e + 1],
                      )

                  mx4 = mxp.tile([128, len(c2_tiles)], f32)
                  for jt, (c0, w) in enumerate(c2_tiles):
                      if jt < 3:
                          # PE wait-carrier: absorbs the h1 (ACT) wait so the
                          # real matmul only waits on its PSUM release (DVE)
                          hc = NTILE * (jt + 1)
                          nc.tensor.matmul(junk, h1[:, hc:hc + 1],
                                           h1[:, hc:hc + 2],
                                           start=True, stop=True,
                                           skip_group_check=True)
                      ps = ps2.tile([128, NTILE], f32)
                      for f in range(FL):
                          nc.tensor.matmul(
                              ps[:, :w],
                              wc2[:, e * FL + f, :],
                              h1[:, c0 + f:c0 + f + w],
                              start=(f == 0),
                              stop=(f == FL - 1),
                          )
                      nc.vector.reduce_max(mx4[:, jt:jt + 1], ps[:, :w],
                                           axis=AX.X)
                  nc.vector.reduce_max(H[:, s:s + 1], mx4, axis=AX.X)

            # --- final linear + dots + softmax (tiny, exact fp32) ---
            # Htan = tanh(H + b2); max commutes with the monotone tanh
            nq = BPC
            Htan = smallp.tile([128, NSEQ], f32)
            nc.scalar.activation(Htan[:, 0:nq], H[:, 0:nq], AFT.Tanh,
                                 bias=b2t[:, 0:1])
            nc.scalar.activation(Htan[:, nq:NSEQ], H[:, nq:NSEQ], AFT.Tanh,
                                 bias=b2t[:, 1:2])
            sps = psf.tile([L, NSEQ], f32, tag="sps")
            nc.tensor.matmul(sps[:, 0:nq], swt[:, 0:L], Htan[:, 0:nq],
                             start=True, stop=True)
            nc.tensor.matmul(sps[:, nq:NSEQ], swt[:, L:128], Htan[:, nq:NSEQ],
                             start=True, stop=True)
            S = smallp.tile([L, NSEQ], f32)
            nc.scalar.activation(S[:, 0:nq], sps[:, 0:nq], AFT.Tanh,
                                 bias=sbt[:, 0:1])
            nc.scalar.activation(S[:, nq:NSEQ], sps[:, nq:NSEQ], AFT.Tanh,
                                 bias=sbt[:, 1:2])

            M = smallp.tile([L, 3 * nq], f32)
            for j in range(3):
                nc.vector.tensor_mul(M[:, j * nq:(j + 1) * nq],
                                     S[:, 0:nq],
                                     S[:, (j + 1) * nq:(j + 2) * nq])
            dps = psf.tile([1, 3 * nq], f32, tag="dps")
            nc.tensor.matmul(dps, ones, M, start=True, stop=True)

            # E = exp(gw * dots) in one ACT op; gb cancels in softmax and
            # the max-subtraction is unnecessary (|dots| <= 64 fits exp fp32)
            E = smallp.tile([1, 3 * nq], f32)
            nc.scalar.activation(E, dps, AFT.Exp, scale=float(gw_val))
            ssum = smallp.tile([1, nq], f32)
            nc.vector.tensor_add(ssum, E[:, 0:nq], E[:, nq:2 * nq])
            nc.vector.tensor_add(ssum, ssum, E[:, 2 * nq:3 * nq])
            rec = smallp.tile([1, nq], f32)
            nc.vector.reciprocal(rec, ssum)
            O = smallp.tile([1, 3 * nq], f32)
            for j in range(3):
                nc.vector.tensor_mul(O[:, j * nq:(j + 1) * nq],
                                     E[:, j * nq:(j + 1) * nq], rec)
            nc.gpsimd.dma_start(out=out_d[:, :], in_=O)

    nc.compile()
    return nc


def _host_prep(q, pos, negs, qw1, qb1, qw2, qb2, qsw, qsb,
               dw1, db1, dw2, db2, dsw, dsb, gw, gb):
    import ml_dtypes
    f = np.float32
    bf = ml_dtypes.bfloat16
    q = np.asarray(q, f)
    pos = np.asarray(pos, f)
    negs = np.asarray(negs, f)

    # per-core activation slabs, pre-transposed to [D, T], cast to bf16
    xall = np.empty((NCORES, NSEQ, D, T), bf)
    xall[:, 0:BPC] = q.reshape(NCORES, BPC, T, D).transpose(0, 1, 3, 2)
    xall[:, BPC:2 * BPC] = pos.reshape(NCORES, BPC, T, D).transpose(0, 1, 3, 2)
    xall[:, 2 * BPC:3 * BPC] = np.asarray(negs[0], f).reshape(
        NCORES, BPC, T, D).transpose(0, 1, 3, 2)
    xall[:, 3 * BPC:4 * BPC] = np.asarray(negs[1], f).reshape(
        NCORES, BPC, T, D).transpose(0, 1, 3, 2)

    # conv weights as lhsT slabs [contract, out]: wcN[e, f] = wN[:, :, f].T
    wc1 = np.empty((128, 2 * FL, 128), bf)    # [d, (e,f), k]
    wc2 = np.empty((128, 2 * FL, 128), bf)
    for fi in range(FL):
        wc1[:, fi] = np.asarray(qw1, f)[:, :, fi].T
        wc1[:, FL + fi] = np.asarray(dw1, f)[:, :, fi].T
        wc2[:, fi] = np.asarray(qw2, f)[:, :, fi].T
        wc2[:, FL + fi] = np.asarray(dw2, f)[:, :, fi].T
    blob = np.zeros((128, 902), f)
    blob[:, 0:64] = np.asarray(qsw, f).T
    blob[:, 64:128] = np.asarray(dsw, f).T
    blob[:, 128] = np.asarray(qb1, f)
    blob[:, 129] = np.asarray(db1, f)
    blob[:, 130] = np.asarray(qb2, f)
    blob[:, 131] = np.asarray(db2, f)
    blob[0:L, 132] = np.asarray(qsb, f)
    blob[0:L, 133] = np.asarray(dsb, f)
    blob[:, 134:518] = np.ascontiguousarray(wc1.reshape(128, 768)).view(
        np.float32)
    blob[:, 518:902] = np.ascontiguousarray(wc2.reshape(128, 768)).view(
        np.float32)

    in_maps = [{"blob": blob, "x": xall[c]} for c in range(NCORES)]
    return in_maps, float(np.asarray(gw, f))


def _assemble(results):
    final = np.empty((B, 3), np.float32)
    for c in range(NCORES):
        o = results[c]["out"][0]              # [3*BPC], j-major
        final[c * BPC:(c + 1) * BPC, :] = o.reshape(3, BPC).T
    return final


def kernel(**inputs):
    global LAST_RESULTS
    from concourse import bass_utils

    in_maps, gw_val = _host_prep(**inputs)
    nc = _build_program(gw_val)
    res = bass_utils.run_bass_kernel_spmd(nc, in_maps, core_ids=list(range(NCORES)))
    LAST_RESULTS = res
    return _assemble(res.results)

